# revision 28
# baseline (speedup 1.0000x reference)
"""GNN message-passing block on 8 Trainium2 NeuronCores.

Strategy (c-sharded, gather-free, fp8 streams):
- Shard pairs by center det (each det owns 32 consecutive pairs; 6250 dets/core).
- The neighbor gather f1[nIdxs] is eliminated: the host expands
  detFeatures[nIdxs] into a dense fp8(e4m3) stream (pure data movement), and
  the device computes f1[n] = relu(W1^T detFn + b1) per pair as a K=128 fp8
  DoubleRow matmul (2 fp8 rows per PE cell -> 0.5 cycles/col).
- pairFeatures also stream as fp8 (halves DMA; z1 pair-term matmul in fp8).
- All other terms of layer-1 accumulate into the same PSUM tile:
    z1 = Wp^T pairF + Wc^T f1[center] (broadcast AP) + Wn^T f1n
- Feature-major layout; 4 pair-tiles of 512 pairs pack into one supertile so
  DVE/ACT run full-width and the PE uses tile_position packing for
  concurrency.  Segment max = strided reduce_max (segments are 32 consecutive
  pairs).  Residual + output round-trip in f16 (ample accuracy headroom).
"""

import sys

sys.path.insert(0, "/opt/trn_rl_repo")

import ml_dtypes
import numpy as np

import concourse.bass as bass
import concourse.tile as tile
from concourse import bacc, mybir
from concourse.bass_utils import run_bass_kernel_spmd

F8 = mybir.dt.float8e4
F16 = mybir.dt.float16
F32 = mybir.dt.float32
NP8 = ml_dtypes.float8_e4m3
DR = mybir.MatmulPerfMode.DoubleRow

N_DETS = 50000
KN = 32
N_CORES = 8
DC_REAL = N_DETS // N_CORES          # 6250 real dets per core
DC = 6272                            # padded dets per core (98 * 64)
S = DC // 64                         # 98 supertiles (64 dets / 2048 pairs each)
PAIRS = DC * KN                      # 200704 padded pairs per core
F1C = S * 16                         # 1568 cols of f1packed
PC = S * 32                          # 3136 pooled cols
PCP = 3584                           # pooled cols padded to 7*512
PT3 = PCP // 512                     # 7 phase-3 tiles
AX = mybir.AxisListType.X
RELU = mybir.ActivationFunctionType.Relu

_CACHE = {}


def _build():
    nc = bacc.Bacc("TRN2", target_bir_lowering=False, debug=False)

    detft16 = nc.dram_tensor("detft16", [128, DC], F16, kind="ExternalInput")
    p8 = nc.dram_tensor("p8", [128, PAIRS // 4], F8, kind="ExternalInput")
    detfn8 = nc.dram_tensor("detfn8", [128, PAIRS], F8, kind="ExternalInput")
    resid16 = nc.dram_tensor("resid16", [128, 2 * PCP], F16, kind="ExternalInput")
    # packed weights: one DMA per dtype (SP dma_start issue is ~565ns each)
    wpk16 = nc.dram_tensor("wpk16", [128, 480], F16, kind="ExternalInput")
    wpk8 = nc.dram_tensor("wpk8", [128, 96], F8, kind="ExternalInput")
    bpk = nc.dram_tensor("bpk", [128, 5], F32, kind="ExternalInput")
    out_t = nc.dram_tensor("out_t", [128, 2 * PCP], F16, kind="ExternalOutput")

    with tile.TileContext(nc) as tc:
        with tc.tile_pool(name="persist", bufs=1) as pp, \
             tc.tile_pool(name="dfn", bufs=3) as dfn_p, \
             tc.tile_pool(name="p8p", bufs=2) as p8_p, \
             tc.tile_pool(name="f1n", bufs=3) as f1n_p, \
             tc.tile_pool(name="hbuf", bufs=4) as h_p, \
             tc.tile_pool(name="ph3", bufs=2) as ph3_p, \
             tc.tile_pool(name="psy", bufs=2, space="PSUM") as psy, \
             tc.tile_pool(name="psz", bufs=1, space="PSUM") as psz, \
             tc.tile_pool(name="psz2", bufs=2, space="PSUM") as psz2:

            # --- load weights / biases / constants; packed, then the first
            # phase-1/phase-2 inputs so the PE starts ASAP.
            wp16_t = pp.tile([128, 480], F16)
            nc.sync.dma_start(wp16_t[:], wpk16[:])
            bp_t = pp.tile([128, 5], F32)
            nc.sync.dma_start(bp_t[:], bpk[:])
            detft_t = pp.tile([128, DC], F16)
            nc.sync.dma_start(detft_t[:, 0:1024], detft16[:, 0:1024])
            wp8_t = pp.tile([128, 96], F8)
            nc.sync.dma_start(wp8_t[:], wpk8[:])

            w1_t = wp16_t[:, 0:32]
            wc4_t = wp16_t[:, 32:96]
            wn4_t = wp16_t[:, 96:160]
            wp1_t = wp16_t[:, 160:224]
            wq0_t = wp16_t[:, 224:288]
            wq1_t = wp16_t[:, 288:352]
            wo_t = wp16_t[:, 352:480]
            w18_t = wp8_t[:, 0:32]
            wp4_t = wp8_t[:, 32:96]
            b1_t = bp_t[:, 0:1]
            bp0_t = bp_t[:, 1:2]
            bp1_t = bp_t[:, 2:3]
            bq0_t = bp_t[:, 3:4]
            bq1_t = bp_t[:, 4:5]

            dfn_tiles = {}
            p8_tiles = {}

            def load_dfn(g):  # tile g covers supertiles {2g, 2g+1}
                t = dfn_p.tile([128, 4096], F8, tag="dfn", name=f"dfn_{g}")
                dw = min(4096, PAIRS - 4096 * g)
                nc.sync.dma_start(t[:, :dw], detfn8[:, 4096 * g:4096 * g + dw])
                dfn_tiles[g] = t

            def load_p8(g):  # group g covers supertiles {4g..4g+3}
                t = p8_p.tile([128, 2048], F8, tag="p8", name=f"p8_{g}")
                sw = min(2048, (PAIRS // 4) - 2048 * g)
                nc.sync.dma_start(t[:, :sw], p8[:, 2048 * g:2048 * g + sw])
                p8_tiles[g] = t

            load_dfn(0)
            load_p8(0)
            load_dfn(1)
            for c in range(1024, DC, 2048):
                nc.sync.dma_start(detft_t[:, c:min(c + 2048, DC)],
                                  detft16[:, c:min(c + 2048, DC)])

            # --- phase 1: f1packed[32q+f, 16s+i] = relu(W1^T detF[64s+16q+i] + b1)
            f1pk = pp.tile([128, F1C], F16)
            chunks = [(0, 256), (256, 256), (512, 512), (1024, 512), (1536, 32)]
            for c0, cn in chunks:
                ps1 = psy.tile([128, 512], F32, tag="ps1")
                ns = cn // 16  # supertiles covered
                s0 = c0 // 16
                dview = detft_t[:].rearrange("p (s g) -> p s g", g=64)
                for q in range(4):
                    rhs = dview[:, s0:s0 + ns, 16 * q:16 * q + 16]
                    nc.tensor.matmul(ps1[32 * q:32 * q + 32, :cn], w1_t[:], rhs,
                                     start=True, stop=True, tile_position=(0, 32 * q))
                nc.scalar.activation(f1pk[:, c0:c0 + cn], ps1[:, :cn], RELU,
                                     bias=b1_t[:], scale=1.0)

            pooled_raw = pp.tile([128, PC], F32)
            pooled = pp.tile([128, PCP], F16)
            nc.vector.memset(pooled[:, PC:PCP], 0.0)

            # --- phase 2: supertiles of 2048 pairs (4 tiles x 512)
            # Software pipelining: f1n for supertile s+1 is computed (PE) and
            # activated (ACT/DVE alternating) while z1 of supertile s and
            # z2/segmax of supertile s-2 stream; phase-3 output tiles are
            # interleaved as soon as their pooled columns complete.
            f1n_tiles = {}
            h1_tiles = {}

            def f1n_stage(s):
                # y = W1^T detFn  (4 col-tiled K=128 fp8 matmuls -> [128, 512])
                dfn_t = dfn_tiles[s // 2]
                do = 2048 * (s % 2)
                ps_y = psy.tile([128, 512], F32, tag="ps1", name=f"psy_{s}")
                for q in range(4):
                    nc.tensor.matmul(ps_y[32 * q:32 * q + 32, :], w18_t[:],
                                     dfn_t[:, do + 512 * q:do + 512 * (q + 1)],
                                     start=True, stop=True, tile_position=(0, 32 * q))
                f1n_t = f1n_p.tile([128, 512], F16, tag="f1n", name=f"f1n_{s}")
                if s % 4 != 3:
                    nc.scalar.activation(f1n_t[:], ps_y[:], RELU, bias=b1_t[:],
                                         scale=1.0)
                else:
                    nc.vector.tensor_scalar(f1n_t[:], ps_y[:], b1_t[:], 0.0,
                                            op0=mybir.AluOpType.add,
                                            op1=mybir.AluOpType.max)
                f1n_tiles[s] = f1n_t

            def emit_l2(sp, h1p):
                # layer 2 + segment max; max(relu(z+b)) == relu(max(z)+b):
                # relu+bias deferred to the pooled array.
                z2 = psz2.tile([128, 1024], F32, tag="z2", name=f"z2_{sp}")
                for q in range(4):
                    hp = 64 * (q % 2)
                    cp = 512 * (q // 2)
                    nc.tensor.matmul(z2[hp:hp + 64, cp:cp + 512],
                                     wp1_t[hp:hp + 64, :],
                                     h1p[hp:hp + 64, cp:cp + 512],
                                     start=True, stop=True, tile_position=(hp, hp))
                src = z2[:].rearrange("p (d k) -> p d k", k=32)
                dst = pooled_raw[:, 32 * sp:32 * sp + 32].rearrange(
                    "p (d one) -> p d one", one=1)
                nc.vector.tensor_reduce(dst, src, op=mybir.AluOpType.max, axis=AX)

            def emit_phase3(t):
                c = 512 * t
                cw = min(512, PC - c)
                nc.scalar.activation(pooled[:, c:c + cw], pooled_raw[:, c:c + cw],
                                     RELU, bias=bp1_t[:], scale=1.0)
                ps_p1 = psy.tile([128, 512], F32, tag="ps1", name=f"p3a_{t}")
                nc.tensor.matmul(ps_p1[0:64, :], wq0_t[0:64, :], pooled[0:64, c:c + 512],
                                 start=True, stop=True, tile_position=(0, 0))
                nc.tensor.matmul(ps_p1[64:128, :], wq0_t[64:128, :], pooled[64:128, c:c + 512],
                                 start=True, stop=True, tile_position=(64, 64))
                p1 = ph3_p.tile([128, 512], F16, tag="p1")
                nc.scalar.activation(p1[:], ps_p1[:], RELU, bias=bq0_t[:], scale=1.0)

                ps_p2 = psy.tile([128, 512], F32, tag="ps1", name=f"p3b_{t}")
                nc.tensor.matmul(ps_p2[0:64, :], wq1_t[0:64, :], p1[0:64, :],
                                 start=True, stop=True, tile_position=(0, 0))
                nc.tensor.matmul(ps_p2[64:128, :], wq1_t[64:128, :], p1[64:128, :],
                                 start=True, stop=True, tile_position=(64, 64))
                p2 = ph3_p.tile([128, 512], F16, tag="p2")
                nc.scalar.activation(p2[:], ps_p2[:], RELU, bias=bq1_t[:], scale=1.0)

                rf = psz2.tile([128, 1024], F32, tag="z2", name=f"rf_{t}")
                nc.tensor.matmul(rf[:, 0:512], wo_t[0:64, :], p2[0:64, :],
                                 start=True, stop=True, tile_position=(0, 0))
                nc.tensor.matmul(rf[:, 512:1024], wo_t[64:128, :], p2[64:128, :],
                                 start=True, stop=True, tile_position=(64, 0))

                res_t = ph3_p.tile([128, 1024], F16, tag="res")
                nc.sync.dma_start(res_t[:], resid16[:, 1024 * t:1024 * (t + 1)])
                o_sb = ph3_p.tile([128, 1024], F16, tag="osb")
                nc.vector.tensor_tensor(o_sb[:], rf[:], res_t[:], op=mybir.AluOpType.add)
                nc.vector.tensor_scalar_max(o_sb[:], o_sb[:], 0.0)
                nc.sync.dma_start(out_t[:, 1024 * t:1024 * (t + 1)], o_sb[:])

            # prologue
            f1n_stage(0)
            for s in range(S):
                if s % 2 == 0 and 2 * (s // 2 + 2) < S:
                    load_dfn(s // 2 + 2)
                if s % 4 == 2 and 4 * (s // 4 + 1) < S:
                    load_p8(s // 4 + 1)
                if s + 1 < S:
                    f1n_stage(s + 1)

                # z1 accumulation: 3 matmuls per quarter, round-robin across quarters
                z1 = psz.tile([128, 1024], F32, tag="z1", name=f"z1_{s}")
                p8_t = p8_tiles[s // 4]
                f1n_t = f1n_tiles.pop(s)
                pcol = 512 * (s % 4)
                for q in range(4):
                    tp = (32 * q, 64 * (q % 2))
                    o = z1[64 * (q % 2):64 * (q % 2) + 64, 512 * (q // 2):512 * (q // 2) + 512]
                    nc.tensor.matmul(o, wp4_t[32 * q:32 * q + 32, :],
                                     p8_t[32 * q:32 * q + 32, pcol:pcol + 512],
                                     start=True, stop=False, tile_position=tp,
                                     skip_group_check=True)
                for q in range(4):
                    tp = (32 * q, 64 * (q % 2))
                    o = z1[64 * (q % 2):64 * (q % 2) + 64, 512 * (q // 2):512 * (q // 2) + 512]
                    rhs = f1pk[32 * q:32 * q + 32, 16 * s:16 * s + 16].rearrange(
                        "p (d one) -> p d one", one=1).to_broadcast([32, 16, 32])
                    nc.tensor.matmul(o, wc4_t[32 * q:32 * q + 32, :], rhs,
                                     start=False, stop=False, tile_position=tp,
                                     skip_group_check=True)
                for q in range(4):
                    tp = (32 * q, 64 * (q % 2))
                    o = z1[64 * (q % 2):64 * (q % 2) + 64, 512 * (q // 2):512 * (q // 2) + 512]
                    nc.tensor.matmul(o, wn4_t[32 * q:32 * q + 32, :],
                                     f1n_t[32 * q:32 * q + 32, :],
                                     start=False, stop=True, tile_position=tp,
                                     skip_group_check=True)

                h1 = h_p.tile([128, 1024], F16, tag="h1", name=f"h1_{s}")
                nc.scalar.activation(h1[:], z1[:], RELU, bias=bp0_t[:], scale=1.0)

                h1_tiles[s] = h1
                if s >= 2:
                    emit_l2(s - 2, h1_tiles.pop(s - 2))
                # phase-3 tile t's pooled cols complete at s = 16t+17; fire at
                # 16t+28 so the ACT-queue wait on segmax has ~11 supertiles of
                # slack (tiles 5/6 drain after the loop).
                if s >= 28 and (s - 28) % 16 == 0 and (s - 28) // 16 <= 4:
                    emit_phase3((s - 28) // 16)
            emit_l2(S - 2, h1_tiles.pop(S - 2))
            emit_l2(S - 1, h1_tiles.pop(S - 1))
            emit_phase3(PT3 - 2)
            emit_phase3(PT3 - 1)

    nc.compile()
    return nc


def _dets_of_core(k):
    return np.arange(DC_REAL * k, DC_REAL * (k + 1))


def _host_prep(detFeatures, cIdxs, nIdxs, pairFeatures,
               W1, b1, Wp0, bp0, Wp1, bp1, Wq0, bq0, Wq1, bq1, Wo, bo):
    """Build per-core input maps. Returns (in_maps, out_perm) where out_perm
    maps device output columns back to det order."""
    f16 = np.float16
    detF = np.asarray(detFeatures, np.float32)
    pairF = np.asarray(pairFeatures, np.float32)
    nI = np.asarray(nIdxs, np.int64)

    # weights (shared across cores), packed per dtype into single tensors
    W1_32 = np.ascontiguousarray(W1, np.float32)
    W1_16 = W1_32.astype(f16)                                          # [128, 32]
    wpk16 = np.concatenate([
        W1_16,
        np.tile(Wp0[32:64].astype(f16), (4, 1)),                       # wc4
        np.tile(Wp0[64:96].astype(f16), (4, 1)),                       # wn4
        np.tile(Wp1.astype(f16), (2, 1)),                              # wp1_2
        np.tile(Wq0.astype(f16), (2, 1)),                              # wq0_2
        np.tile(Wq1.astype(f16), (2, 1)),                              # wq1_2
        np.tile(Wo.astype(f16), (2, 1)),                               # wo2
    ], axis=1)                                                         # [128, 480]
    wpk8 = np.concatenate([
        W1_32.astype(NP8),
        np.tile(Wp0[0:32].astype(NP8), (4, 1)),                        # wp4
    ], axis=1)                                                         # [128, 96]
    bpk = np.stack([
        np.tile(np.asarray(b1, np.float32), 4),
        np.tile(np.asarray(bp0, np.float32), 2),
        np.tile(np.asarray(bp1, np.float32), 2),
        np.tile(np.asarray(bq0, np.float32), 2),
        np.tile(np.asarray(bq1, np.float32), 2),
    ], axis=1)                                                         # [128, 5]
    bo32 = np.asarray(bo, np.float32)

    # det-order scramble for pooled/output columns:
    # local det d: s = d//64, q = (d%64)//16, i = d%16
    d = np.arange(DC)
    s_, q_, i_ = d // 64, (d % 64) // 16, d % 16
    pooled_col = 32 * s_ + 16 * (q_ // 2) + i_
    half = q_ % 2
    t3 = pooled_col // 512
    out_col = 1024 * t3 + 512 * half + (pooled_col % 512)              # [DC]

    detF8u = detF.astype(NP8).view(np.uint8)                           # [N, 128]

    in_maps = []
    for k in range(N_CORES):
        dets = _dets_of_core(k)
        dloc = detF[dets]                                              # [6250, 128]
        dpad = np.zeros((DC, 128), np.float32)
        dpad[:DC_REAL] = dloc
        detft16 = np.ascontiguousarray(dpad.T.astype(f16))             # [128, DC]

        # resid16[:, out_col[d]] = detF[d] + bo  (scrambled; pads zero)
        resid = np.zeros((2 * PCP, 128), np.float32)
        resid[out_col[:DC_REAL]] = dloc + bo32
        resid16 = np.ascontiguousarray(resid.T.astype(f16))            # [128, 2*PCP]

        # pairs of this core, padded; fp8 strip packing as in the f16 version
        pf = np.zeros((PAIRS, 32), NP8)
        pf[:DC_REAL * KN] = pairF[DC_REAL * KN * k: DC_REAL * KN * (k + 1)].astype(NP8)
        p8 = np.ascontiguousarray(
            pf.reshape(S, 4, 512, 32).transpose(1, 3, 0, 2).reshape(128, S * 512)
        )

        ni = np.zeros(PAIRS, np.int64)
        ni[:DC_REAL * KN] = nI[DC_REAL * KN * k: DC_REAL * KN * (k + 1)]
        detfn8 = np.ascontiguousarray(detF8u[ni].T).view(NP8)          # [128, PAIRS]

        in_maps.append({
            "detft16": detft16, "p8": p8.view(NP8), "detfn8": detfn8,
            "resid16": resid16,
            "wpk16": wpk16, "wpk8": wpk8, "bpk": bpk,
        })
    return in_maps, out_col


def _run(inputs, trace=False):
    if "nc" not in _CACHE:
        _CACHE["nc"] = _build()
    nc = _CACHE["nc"]
    in_maps, out_col = _host_prep(**inputs)
    res = run_bass_kernel_spmd(nc, in_maps, core_ids=list(range(N_CORES)),
                               trace=trace)
    outs = []
    for k in range(N_CORES):
        ot = res.results[k]["out_t"]                                   # [128, 2*PCP]
        outs.append(ot[:, out_col[:DC_REAL]].T)                        # [6250, 128]
    full = np.concatenate(outs, axis=0).astype(np.float32)
    return full, res


def kernel(**inputs):
    inputs = {k: np.asarray(v) for k, v in inputs.items()}
    full, _ = _run(inputs, trace=False)
    return full


# revision 30
# speedup vs baseline: 1.0582x; 1.0582x over previous
"""GNN message-passing block on 8 Trainium2 NeuronCores.

Strategy (c-sharded, gather-free, fp8 streams):
- Shard pairs by center det (each det owns 32 consecutive pairs; 6250 dets/core).
- The neighbor gather f1[nIdxs] is eliminated: the host expands
  detFeatures[nIdxs] into a dense fp8(e4m3) stream (pure data movement), and
  the device computes f1[n] = relu(W1^T detFn + b1) per pair as a K=128 fp8
  DoubleRow matmul (2 fp8 rows per PE cell -> 0.5 cycles/col).
- pairFeatures also stream as fp8 (halves DMA; z1 pair-term matmul in fp8).
- All other terms of layer-1 accumulate into the same PSUM tile:
    z1 = Wp^T pairF + Wc^T f1[center] (broadcast AP) + Wn^T f1n
- Feature-major layout; 4 pair-tiles of 512 pairs pack into one supertile so
  DVE/ACT run full-width and the PE uses tile_position packing for
  concurrency.  Segment max = strided reduce_max (segments are 32 consecutive
  pairs).  Residual + output round-trip in f16 (ample accuracy headroom).
"""

import sys

sys.path.insert(0, "/opt/trn_rl_repo")

import ml_dtypes
import numpy as np

import concourse.bass as bass
import concourse.tile as tile
from concourse import bacc, mybir
from concourse.bass_utils import run_bass_kernel_spmd

F8 = mybir.dt.float8e4
F16 = mybir.dt.float16
F32 = mybir.dt.float32
NP8 = ml_dtypes.float8_e4m3
DR = mybir.MatmulPerfMode.DoubleRow

N_DETS = 50000
KN = 32
N_CORES = 8
DC_REAL = N_DETS // N_CORES          # 6250 real dets per core
DC = 6272                            # padded dets per core (98 * 64)
S = DC // 64                         # 98 supertiles (64 dets / 2048 pairs each)
PAIRS = DC * KN                      # 200704 padded pairs per core
F1C = S * 16                         # 1568 cols of f1packed
PC = S * 32                          # 3136 pooled cols
PCP = 3584                           # pooled cols padded to 7*512
PT3 = PCP // 512                     # 7 phase-3 tiles
AX = mybir.AxisListType.X
RELU = mybir.ActivationFunctionType.Relu

_CACHE = {}


def _build():
    nc = bacc.Bacc("TRN2", target_bir_lowering=False, debug=False)

    detft16 = nc.dram_tensor("detft16", [128, DC], F16, kind="ExternalInput")
    p8 = nc.dram_tensor("p8", [128, PAIRS // 4], F8, kind="ExternalInput")
    detfn8 = nc.dram_tensor("detfn8", [128, PAIRS], F8, kind="ExternalInput")
    resid16 = nc.dram_tensor("resid16", [128, 2 * PCP], F16, kind="ExternalInput")
    # packed weights: one DMA per dtype (SP dma_start issue is ~565ns each)
    wpk16 = nc.dram_tensor("wpk16", [128, 480], F16, kind="ExternalInput")
    wpk8 = nc.dram_tensor("wpk8", [128, 96], F8, kind="ExternalInput")
    bpk = nc.dram_tensor("bpk", [128, 5], F32, kind="ExternalInput")
    out_t = nc.dram_tensor("out_t", [128, 2 * PCP], F16, kind="ExternalOutput")

    with tile.TileContext(nc) as tc:
        with tc.tile_pool(name="persist", bufs=1) as pp, \
             tc.tile_pool(name="dfn", bufs=3) as dfn_p, \
             tc.tile_pool(name="p8p", bufs=2) as p8_p, \
             tc.tile_pool(name="f1n", bufs=3) as f1n_p, \
             tc.tile_pool(name="hbuf", bufs=4) as h_p, \
             tc.tile_pool(name="ph3", bufs=2) as ph3_p, \
             tc.tile_pool(name="psy", bufs=2, space="PSUM") as psy, \
             tc.tile_pool(name="psz", bufs=2, space="PSUM") as psz, \
             tc.tile_pool(name="psz2", bufs=1, space="PSUM") as psz2:

            # --- load weights / biases / constants; packed, then the first
            # phase-1/phase-2 inputs so the PE starts ASAP.
            wp16_t = pp.tile([128, 480], F16)
            nc.sync.dma_start(wp16_t[:], wpk16[:])
            bp_t = pp.tile([128, 5], F32)
            nc.sync.dma_start(bp_t[:], bpk[:])
            detft_t = pp.tile([128, DC], F16)
            nc.sync.dma_start(detft_t[:, 0:1024], detft16[:, 0:1024])
            wp8_t = pp.tile([128, 96], F8)
            nc.sync.dma_start(wp8_t[:], wpk8[:])

            w1_t = wp16_t[:, 0:32]
            wc4_t = wp16_t[:, 32:96]
            wn4_t = wp16_t[:, 96:160]
            wp1_t = wp16_t[:, 160:224]
            wq0_t = wp16_t[:, 224:288]
            wq1_t = wp16_t[:, 288:352]
            wo_t = wp16_t[:, 352:480]
            w18_t = wp8_t[:, 0:32]
            wp4_t = wp8_t[:, 32:96]
            b1_t = bp_t[:, 0:1]
            bp0_t = bp_t[:, 1:2]
            bp1_t = bp_t[:, 2:3]
            bq0_t = bp_t[:, 3:4]
            bq1_t = bp_t[:, 4:5]

            dfn_tiles = {}
            p8_tiles = {}

            def load_dfn(g):  # tile g covers supertiles {2g, 2g+1}
                t = dfn_p.tile([128, 4096], F8, tag="dfn", name=f"dfn_{g}")
                dw = min(4096, PAIRS - 4096 * g)
                nc.sync.dma_start(t[:, :dw], detfn8[:, 4096 * g:4096 * g + dw])
                dfn_tiles[g] = t

            def load_p8(g):  # group g covers supertiles {4g..4g+3}
                t = p8_p.tile([128, 2048], F8, tag="p8", name=f"p8_{g}")
                sw = min(2048, (PAIRS // 4) - 2048 * g)
                nc.sync.dma_start(t[:, :sw], p8[:, 2048 * g:2048 * g + sw])
                p8_tiles[g] = t

            load_dfn(0)
            load_p8(0)
            load_dfn(1)
            for c in range(1024, DC, 2048):
                nc.sync.dma_start(detft_t[:, c:min(c + 2048, DC)],
                                  detft16[:, c:min(c + 2048, DC)])

            # --- phase 1: f1packed[32q+f, 16s+i] = relu(W1^T detF[64s+16q+i] + b1)
            f1pk = pp.tile([128, F1C], F16)
            chunks = [(0, 256), (256, 256), (512, 512), (1024, 512), (1536, 32)]
            for c0, cn in chunks:
                ps1 = psy.tile([128, 512], F32, tag="ps1")
                ns = cn // 16  # supertiles covered
                s0 = c0 // 16
                dview = detft_t[:].rearrange("p (s g) -> p s g", g=64)
                for q in range(4):
                    rhs = dview[:, s0:s0 + ns, 16 * q:16 * q + 16]
                    nc.tensor.matmul(ps1[32 * q:32 * q + 32, :cn], w1_t[:], rhs,
                                     start=True, stop=True, tile_position=(0, 32 * q))
                nc.scalar.activation(f1pk[:, c0:c0 + cn], ps1[:, :cn], RELU,
                                     bias=b1_t[:], scale=1.0)

            pooled_raw = pp.tile([128, PC], F32)
            pooled = pp.tile([128, PCP], F16)
            nc.vector.memset(pooled[:, PC:PCP], 0.0)

            # --- phase 2: supertiles of 2048 pairs (4 tiles x 512)
            # Software pipelining: f1n for supertile s+1 is computed (PE) and
            # activated (ACT/DVE alternating) while z1 of supertile s and
            # z2/segmax of supertile s-2 stream; phase-3 output tiles are
            # interleaved as soon as their pooled columns complete.
            f1n_tiles = {}
            h1_tiles = {}

            def f1n_stage(s):
                # y = W1^T detFn  (4 col-tiled K=128 fp8 matmuls -> [128, 512])
                dfn_t = dfn_tiles[s // 2]
                do = 2048 * (s % 2)
                ps_y = psy.tile([128, 512], F32, tag="ps1", name=f"psy_{s}")
                for q in range(4):
                    nc.tensor.matmul(ps_y[32 * q:32 * q + 32, :], w18_t[:],
                                     dfn_t[:, do + 512 * q:do + 512 * (q + 1)],
                                     start=True, stop=True, tile_position=(0, 32 * q))
                f1n_t = f1n_p.tile([128, 512], F16, tag="f1n", name=f"f1n_{s}")
                if s % 4 != 3:
                    nc.scalar.activation(f1n_t[:], ps_y[:], RELU, bias=b1_t[:],
                                         scale=1.0)
                else:
                    nc.vector.tensor_scalar(f1n_t[:], ps_y[:], b1_t[:], 0.0,
                                            op0=mybir.AluOpType.add,
                                            op1=mybir.AluOpType.max)
                f1n_tiles[s] = f1n_t

            def emit_l2(sp, h1p):
                # layer 2 + segment max; max(relu(z+b)) == relu(max(z)+b):
                # relu+bias deferred to the pooled array.
                z2 = psz2.tile([128, 1024], F32, tag="z2", name=f"z2_{sp}")
                for q in range(4):
                    hp = 64 * (q % 2)
                    cp = 512 * (q // 2)
                    nc.tensor.matmul(z2[hp:hp + 64, cp:cp + 512],
                                     wp1_t[hp:hp + 64, :],
                                     h1p[hp:hp + 64, cp:cp + 512],
                                     start=True, stop=True, tile_position=(hp, hp))
                src = z2[:].rearrange("p (d k) -> p d k", k=32)
                dst = pooled_raw[:, 32 * sp:32 * sp + 32].rearrange(
                    "p (d one) -> p d one", one=1)
                nc.vector.tensor_reduce(dst, src, op=mybir.AluOpType.max, axis=AX)

            def emit_phase3(t):
                c = 512 * t
                cw = min(512, PC - c)
                nc.scalar.activation(pooled[:, c:c + cw], pooled_raw[:, c:c + cw],
                                     RELU, bias=bp1_t[:], scale=1.0)
                ps_p1 = psy.tile([128, 512], F32, tag="ps1", name=f"p3a_{t}")
                nc.tensor.matmul(ps_p1[0:64, :], wq0_t[0:64, :], pooled[0:64, c:c + 512],
                                 start=True, stop=True, tile_position=(0, 0))
                nc.tensor.matmul(ps_p1[64:128, :], wq0_t[64:128, :], pooled[64:128, c:c + 512],
                                 start=True, stop=True, tile_position=(64, 64))
                p1 = ph3_p.tile([128, 512], F16, tag="p1")
                nc.scalar.activation(p1[:], ps_p1[:], RELU, bias=bq0_t[:], scale=1.0)

                ps_p2 = psy.tile([128, 512], F32, tag="ps1", name=f"p3b_{t}")
                nc.tensor.matmul(ps_p2[0:64, :], wq1_t[0:64, :], p1[0:64, :],
                                 start=True, stop=True, tile_position=(0, 0))
                nc.tensor.matmul(ps_p2[64:128, :], wq1_t[64:128, :], p1[64:128, :],
                                 start=True, stop=True, tile_position=(64, 64))
                p2 = ph3_p.tile([128, 512], F16, tag="p2")
                nc.scalar.activation(p2[:], ps_p2[:], RELU, bias=bq1_t[:], scale=1.0)

                rf = psz.tile([128, 1024], F32, tag="z1", name=f"rf_{t}")
                nc.tensor.matmul(rf[:, 0:512], wo_t[0:64, :], p2[0:64, :],
                                 start=True, stop=True, tile_position=(0, 0))
                nc.tensor.matmul(rf[:, 512:1024], wo_t[64:128, :], p2[64:128, :],
                                 start=True, stop=True, tile_position=(64, 0))

                res_t = ph3_p.tile([128, 1024], F16, tag="res")
                nc.sync.dma_start(res_t[:], resid16[:, 1024 * t:1024 * (t + 1)])
                o_sb = ph3_p.tile([128, 1024], F16, tag="osb")
                nc.vector.tensor_tensor(o_sb[:], rf[:], res_t[:], op=mybir.AluOpType.add)
                nc.vector.tensor_scalar_max(o_sb[:], o_sb[:], 0.0)
                nc.sync.dma_start(out_t[:, 1024 * t:1024 * (t + 1)], o_sb[:])

            # prologue
            f1n_stage(0)
            for s in range(S):
                if s % 2 == 0 and 2 * (s // 2 + 2) < S:
                    load_dfn(s // 2 + 2)
                if s % 4 == 2 and 4 * (s // 4 + 1) < S:
                    load_p8(s // 4 + 1)
                if s + 1 < S:
                    f1n_stage(s + 1)

                # z1 accumulation: 3 matmuls per quarter, round-robin across quarters
                z1 = psz.tile([128, 1024], F32, tag="z1", name=f"z1_{s}")
                p8_t = p8_tiles[s // 4]
                f1n_t = f1n_tiles.pop(s)
                pcol = 512 * (s % 4)
                for q in range(4):
                    tp = (32 * q, 64 * (q % 2))
                    o = z1[64 * (q % 2):64 * (q % 2) + 64, 512 * (q // 2):512 * (q // 2) + 512]
                    nc.tensor.matmul(o, wp4_t[32 * q:32 * q + 32, :],
                                     p8_t[32 * q:32 * q + 32, pcol:pcol + 512],
                                     start=True, stop=False, tile_position=tp,
                                     skip_group_check=True)
                for q in range(4):
                    tp = (32 * q, 64 * (q % 2))
                    o = z1[64 * (q % 2):64 * (q % 2) + 64, 512 * (q // 2):512 * (q // 2) + 512]
                    rhs = f1pk[32 * q:32 * q + 32, 16 * s:16 * s + 16].rearrange(
                        "p (d one) -> p d one", one=1).to_broadcast([32, 16, 32])
                    nc.tensor.matmul(o, wc4_t[32 * q:32 * q + 32, :], rhs,
                                     start=False, stop=False, tile_position=tp,
                                     skip_group_check=True)
                for q in range(4):
                    tp = (32 * q, 64 * (q % 2))
                    o = z1[64 * (q % 2):64 * (q % 2) + 64, 512 * (q // 2):512 * (q // 2) + 512]
                    nc.tensor.matmul(o, wn4_t[32 * q:32 * q + 32, :],
                                     f1n_t[32 * q:32 * q + 32, :],
                                     start=False, stop=True, tile_position=tp,
                                     skip_group_check=True)

                h1 = h_p.tile([128, 1024], F16, tag="h1", name=f"h1_{s}")
                nc.scalar.activation(h1[:], z1[:], RELU, bias=bp0_t[:], scale=1.0)

                h1_tiles[s] = h1
                if s >= 2:
                    emit_l2(s - 2, h1_tiles.pop(s - 2))
                # phase-3 tile t's pooled cols complete at s = 16t+17; fire at
                # 16t+28 so the ACT-queue wait on segmax has ~11 supertiles of
                # slack (tiles 5/6 drain after the loop).
                if s >= 28 and (s - 28) % 16 == 0 and (s - 28) // 16 <= 4:
                    emit_phase3((s - 28) // 16)
            emit_l2(S - 2, h1_tiles.pop(S - 2))
            emit_l2(S - 1, h1_tiles.pop(S - 1))
            emit_phase3(PT3 - 2)
            emit_phase3(PT3 - 1)

    nc.compile()
    return nc


def _dets_of_core(k):
    return np.arange(DC_REAL * k, DC_REAL * (k + 1))


def _host_prep(detFeatures, cIdxs, nIdxs, pairFeatures,
               W1, b1, Wp0, bp0, Wp1, bp1, Wq0, bq0, Wq1, bq1, Wo, bo):
    """Build per-core input maps. Returns (in_maps, out_perm) where out_perm
    maps device output columns back to det order."""
    f16 = np.float16
    detF = np.asarray(detFeatures, np.float32)
    pairF = np.asarray(pairFeatures, np.float32)
    nI = np.asarray(nIdxs, np.int64)

    # weights (shared across cores), packed per dtype into single tensors
    W1_32 = np.ascontiguousarray(W1, np.float32)
    W1_16 = W1_32.astype(f16)                                          # [128, 32]
    wpk16 = np.concatenate([
        W1_16,
        np.tile(Wp0[32:64].astype(f16), (4, 1)),                       # wc4
        np.tile(Wp0[64:96].astype(f16), (4, 1)),                       # wn4
        np.tile(Wp1.astype(f16), (2, 1)),                              # wp1_2
        np.tile(Wq0.astype(f16), (2, 1)),                              # wq0_2
        np.tile(Wq1.astype(f16), (2, 1)),                              # wq1_2
        np.tile(Wo.astype(f16), (2, 1)),                               # wo2
    ], axis=1)                                                         # [128, 480]
    wpk8 = np.concatenate([
        W1_32.astype(NP8),
        np.tile(Wp0[0:32].astype(NP8), (4, 1)),                        # wp4
    ], axis=1)                                                         # [128, 96]
    bpk = np.stack([
        np.tile(np.asarray(b1, np.float32), 4),
        np.tile(np.asarray(bp0, np.float32), 2),
        np.tile(np.asarray(bp1, np.float32), 2),
        np.tile(np.asarray(bq0, np.float32), 2),
        np.tile(np.asarray(bq1, np.float32), 2),
    ], axis=1)                                                         # [128, 5]
    bo32 = np.asarray(bo, np.float32)

    # det-order scramble for pooled/output columns:
    # local det d: s = d//64, q = (d%64)//16, i = d%16
    d = np.arange(DC)
    s_, q_, i_ = d // 64, (d % 64) // 16, d % 16
    pooled_col = 32 * s_ + 16 * (q_ // 2) + i_
    half = q_ % 2
    t3 = pooled_col // 512
    out_col = 1024 * t3 + 512 * half + (pooled_col % 512)              # [DC]

    detF8u = detF.astype(NP8).view(np.uint8)                           # [N, 128]

    in_maps = []
    for k in range(N_CORES):
        dets = _dets_of_core(k)
        dloc = detF[dets]                                              # [6250, 128]
        dpad = np.zeros((DC, 128), np.float32)
        dpad[:DC_REAL] = dloc
        detft16 = np.ascontiguousarray(dpad.T.astype(f16))             # [128, DC]

        # resid16[:, out_col[d]] = detF[d] + bo  (scrambled; pads zero)
        resid = np.zeros((2 * PCP, 128), np.float32)
        resid[out_col[:DC_REAL]] = dloc + bo32
        resid16 = np.ascontiguousarray(resid.T.astype(f16))            # [128, 2*PCP]

        # pairs of this core, padded; fp8 strip packing as in the f16 version
        pf = np.zeros((PAIRS, 32), NP8)
        pf[:DC_REAL * KN] = pairF[DC_REAL * KN * k: DC_REAL * KN * (k + 1)].astype(NP8)
        p8 = np.ascontiguousarray(
            pf.reshape(S, 4, 512, 32).transpose(1, 3, 0, 2).reshape(128, S * 512)
        )

        ni = np.zeros(PAIRS, np.int64)
        ni[:DC_REAL * KN] = nI[DC_REAL * KN * k: DC_REAL * KN * (k + 1)]
        detfn8 = np.ascontiguousarray(detF8u[ni].T).view(NP8)          # [128, PAIRS]

        in_maps.append({
            "detft16": detft16, "p8": p8.view(NP8), "detfn8": detfn8,
            "resid16": resid16,
            "wpk16": wpk16, "wpk8": wpk8, "bpk": bpk,
        })
    return in_maps, out_col


def _run(inputs, trace=False):
    if "nc" not in _CACHE:
        _CACHE["nc"] = _build()
    nc = _CACHE["nc"]
    in_maps, out_col = _host_prep(**inputs)
    res = run_bass_kernel_spmd(nc, in_maps, core_ids=list(range(N_CORES)),
                               trace=trace)
    outs = []
    for k in range(N_CORES):
        ot = res.results[k]["out_t"]                                   # [128, 2*PCP]
        outs.append(ot[:, out_col[:DC_REAL]].T)                        # [6250, 128]
    full = np.concatenate(outs, axis=0).astype(np.float32)
    return full, res


def kernel(**inputs):
    inputs = {k: np.asarray(v) for k, v in inputs.items()}
    full, _ = _run(inputs, trace=False)
    return full


# revision 32
# speedup vs baseline: 1.0609x; 1.0025x over previous
"""GNN message-passing block on 8 Trainium2 NeuronCores.

Strategy (c-sharded, gather-free, fp8 streams):
- Shard pairs by center det (each det owns 32 consecutive pairs; 6250 dets/core).
- The neighbor gather f1[nIdxs] is eliminated: the host expands
  detFeatures[nIdxs] into a dense fp8(e4m3) stream (pure data movement), and
  the device computes f1[n] = relu(W1^T detFn + b1) per pair as a K=128 fp8
  DoubleRow matmul (2 fp8 rows per PE cell -> 0.5 cycles/col).
- pairFeatures also stream as fp8 (halves DMA; z1 pair-term matmul in fp8).
- All other terms of layer-1 accumulate into the same PSUM tile:
    z1 = Wp^T pairF + Wc^T f1[center] (broadcast AP) + Wn^T f1n
- Feature-major layout; 4 pair-tiles of 512 pairs pack into one supertile so
  DVE/ACT run full-width and the PE uses tile_position packing for
  concurrency.  Segment max = strided reduce_max (segments are 32 consecutive
  pairs).  Residual + output round-trip in f16 (ample accuracy headroom).
"""

import sys

sys.path.insert(0, "/opt/trn_rl_repo")

import ml_dtypes
import numpy as np

import concourse.bass as bass
import concourse.tile as tile
from concourse import bacc, mybir
from concourse.bass_utils import run_bass_kernel_spmd

F8 = mybir.dt.float8e4
F16 = mybir.dt.float16
F32 = mybir.dt.float32
NP8 = ml_dtypes.float8_e4m3
DR = mybir.MatmulPerfMode.DoubleRow

N_DETS = 50000
KN = 32
N_CORES = 8
DC_REAL = N_DETS // N_CORES          # 6250 real dets per core
DC = 6272                            # padded dets per core (98 * 64)
S = DC // 64                         # 98 supertiles (64 dets / 2048 pairs each)
PAIRS = DC * KN                      # 200704 padded pairs per core
F1C = S * 16                         # 1568 cols of f1packed
PC = S * 32                          # 3136 pooled cols
PCP = 3584                           # pooled cols padded to 7*512
PT3 = PCP // 512                     # 7 phase-3 tiles
AX = mybir.AxisListType.X
RELU = mybir.ActivationFunctionType.Relu

_CACHE = {}


def _build():
    nc = bacc.Bacc("TRN2", target_bir_lowering=False, debug=False)

    detft16 = nc.dram_tensor("detft16", [128, DC], F16, kind="ExternalInput")
    p8 = nc.dram_tensor("p8", [128, PAIRS // 4], F8, kind="ExternalInput")
    detfn8 = nc.dram_tensor("detfn8", [128, PAIRS], F8, kind="ExternalInput")
    resid16 = nc.dram_tensor("resid16", [128, 2 * PCP], F16, kind="ExternalInput")
    # packed weights: one DMA per dtype (SP dma_start issue is ~565ns each)
    wpk16 = nc.dram_tensor("wpk16", [128, 480], F16, kind="ExternalInput")
    wpk8 = nc.dram_tensor("wpk8", [128, 96], F8, kind="ExternalInput")
    bpk = nc.dram_tensor("bpk", [128, 5], F32, kind="ExternalInput")
    out_t = nc.dram_tensor("out_t", [128, 2 * PCP], F16, kind="ExternalOutput")

    with tile.TileContext(nc) as tc:
        with tc.tile_pool(name="persist", bufs=1) as pp, \
             tc.tile_pool(name="dfn", bufs=3) as dfn_p, \
             tc.tile_pool(name="p8p", bufs=2) as p8_p, \
             tc.tile_pool(name="f1n", bufs=3) as f1n_p, \
             tc.tile_pool(name="hbuf", bufs=4) as h_p, \
             tc.tile_pool(name="ph3", bufs=2) as ph3_p, \
             tc.tile_pool(name="psy", bufs=2, space="PSUM") as psy, \
             tc.tile_pool(name="psz", bufs=2, space="PSUM") as psz, \
             tc.tile_pool(name="psz2", bufs=1, space="PSUM") as psz2:

            # --- load weights / biases / constants; packed, then the first
            # phase-1/phase-2 inputs so the PE starts ASAP.
            wp16_t = pp.tile([128, 480], F16)
            nc.sync.dma_start(wp16_t[:], wpk16[:])
            bp_t = pp.tile([128, 5], F32)
            nc.sync.dma_start(bp_t[:], bpk[:])
            detft_t = pp.tile([128, DC], F16)
            nc.sync.dma_start(detft_t[:, 0:1024], detft16[:, 0:1024])
            wp8_t = pp.tile([128, 96], F8)
            nc.sync.dma_start(wp8_t[:], wpk8[:])

            w1_t = wp16_t[:, 0:32]
            wc4_t = wp16_t[:, 32:96]
            wn4_t = wp16_t[:, 96:160]
            wp1_t = wp16_t[:, 160:224]
            wq0_t = wp16_t[:, 224:288]
            wq1_t = wp16_t[:, 288:352]
            wo_t = wp16_t[:, 352:480]
            w18_t = wp8_t[:, 0:32]
            wp4_t = wp8_t[:, 32:96]
            b1_t = bp_t[:, 0:1]
            bp0_t = bp_t[:, 1:2]
            bp1_t = bp_t[:, 2:3]
            bq0_t = bp_t[:, 3:4]
            bq1_t = bp_t[:, 4:5]

            dfn_tiles = {}
            p8_tiles = {}

            def load_dfn(g):  # tile g covers supertiles {2g, 2g+1}
                t = dfn_p.tile([128, 4096], F8, tag="dfn", name=f"dfn_{g}")
                dw = min(4096, PAIRS - 4096 * g)
                nc.sync.dma_start(t[:, :dw], detfn8[:, 4096 * g:4096 * g + dw])
                dfn_tiles[g] = t

            def load_p8(g):  # group g covers supertiles {4g..4g+3}
                t = p8_p.tile([128, 2048], F8, tag="p8", name=f"p8_{g}")
                sw = min(2048, (PAIRS // 4) - 2048 * g)
                nc.sync.dma_start(t[:, :sw], p8[:, 2048 * g:2048 * g + sw])
                p8_tiles[g] = t

            load_dfn(0)
            load_p8(0)
            load_dfn(1)
            for c in range(1024, DC, 2048):
                nc.sync.dma_start(detft_t[:, c:min(c + 2048, DC)],
                                  detft16[:, c:min(c + 2048, DC)])

            # --- phase 1: f1packed[32q+f, 16s+i] = relu(W1^T detF[64s+16q+i] + b1)
            f1pk = pp.tile([128, F1C], F16)
            chunks = [(0, 256), (256, 256), (512, 512), (1024, 512), (1536, 32)]
            for c0, cn in chunks:
                ps1 = psy.tile([128, 512], F32, tag="ps1")
                ns = cn // 16  # supertiles covered
                s0 = c0 // 16
                dview = detft_t[:].rearrange("p (s g) -> p s g", g=64)
                for q in range(4):
                    rhs = dview[:, s0:s0 + ns, 16 * q:16 * q + 16]
                    nc.tensor.matmul(ps1[32 * q:32 * q + 32, :cn], w1_t[:], rhs,
                                     start=True, stop=True, tile_position=(0, 32 * q))
                nc.scalar.activation(f1pk[:, c0:c0 + cn], ps1[:, :cn], RELU,
                                     bias=b1_t[:], scale=1.0)

            pooled_raw = pp.tile([128, PC], F32)
            pooled = pp.tile([128, PCP], F16)
            nc.vector.memset(pooled[:, PC:PCP], 0.0)

            # --- phase 2: supertiles of 2048 pairs (4 tiles x 512)
            # Software pipelining: f1n for supertile s+1 is computed (PE) and
            # activated (ACT/DVE alternating) while z1 of supertile s and
            # z2/segmax of supertile s-2 stream; phase-3 output tiles are
            # interleaved as soon as their pooled columns complete.
            f1n_tiles = {}
            h1_tiles = {}

            def f1n_stage(s):
                # y = W1^T detFn  (4 col-tiled K=128 fp8 matmuls -> [128, 512])
                dfn_t = dfn_tiles[s // 2]
                do = 2048 * (s % 2)
                ps_y = psy.tile([128, 512], F32, tag="ps1", name=f"psy_{s}")
                for q in range(4):
                    nc.tensor.matmul(ps_y[32 * q:32 * q + 32, :], w18_t[:],
                                     dfn_t[:, do + 512 * q:do + 512 * (q + 1)],
                                     start=True, stop=True, tile_position=(0, 32 * q))
                f1n_t = f1n_p.tile([128, 512], F16, tag="f1n", name=f"f1n_{s}")
                if s % 4 != 3:
                    nc.scalar.activation(f1n_t[:], ps_y[:], RELU, bias=b1_t[:],
                                         scale=1.0)
                else:
                    nc.vector.tensor_scalar(f1n_t[:], ps_y[:], b1_t[:], 0.0,
                                            op0=mybir.AluOpType.add,
                                            op1=mybir.AluOpType.max)
                f1n_tiles[s] = f1n_t

            def emit_l2(sp, h1p):
                # layer 2 + segment max; max(relu(z+b)) == relu(max(z)+b):
                # relu+bias deferred to the pooled array.
                z2 = psz2.tile([128, 1024], F32, tag="z2", name=f"z2_{sp}")
                for q in range(4):
                    hp = 64 * (q % 2)
                    cp = 512 * (q // 2)
                    nc.tensor.matmul(z2[hp:hp + 64, cp:cp + 512],
                                     wp1_t[hp:hp + 64, :],
                                     h1p[hp:hp + 64, cp:cp + 512],
                                     start=True, stop=True, tile_position=(hp, hp))
                src = z2[:].rearrange("p (d k) -> p d k", k=32)
                dst = pooled_raw[:, 32 * sp:32 * sp + 32].rearrange(
                    "p (d one) -> p d one", one=1)
                nc.vector.tensor_reduce(dst, src, op=mybir.AluOpType.max, axis=AX)

            p3_state = {}

            def emit_p3a(t):
                # phase-3 compute: pooled act slice, 2-layer MLP, output FC
                c = 512 * t
                cw = min(512, PC - c)
                nc.scalar.activation(pooled[:, c:c + cw], pooled_raw[:, c:c + cw],
                                     RELU, bias=bp1_t[:], scale=1.0)
                ps_p1 = psy.tile([128, 512], F32, tag="ps1", name=f"p3a_{t}")
                nc.tensor.matmul(ps_p1[0:64, :], wq0_t[0:64, :], pooled[0:64, c:c + 512],
                                 start=True, stop=True, tile_position=(0, 0))
                nc.tensor.matmul(ps_p1[64:128, :], wq0_t[64:128, :], pooled[64:128, c:c + 512],
                                 start=True, stop=True, tile_position=(64, 64))
                p1 = ph3_p.tile([128, 512], F16, tag="p1")
                nc.scalar.activation(p1[:], ps_p1[:], RELU, bias=bq0_t[:], scale=1.0)

                ps_p2 = psy.tile([128, 512], F32, tag="ps1", name=f"p3b_{t}")
                nc.tensor.matmul(ps_p2[0:64, :], wq1_t[0:64, :], p1[0:64, :],
                                 start=True, stop=True, tile_position=(0, 0))
                nc.tensor.matmul(ps_p2[64:128, :], wq1_t[64:128, :], p1[64:128, :],
                                 start=True, stop=True, tile_position=(64, 64))
                p2 = ph3_p.tile([128, 512], F16, tag="p2")
                nc.scalar.activation(p2[:], ps_p2[:], RELU, bias=bq1_t[:], scale=1.0)

                rf = psz.tile([128, 1024], F32, tag="z1", name=f"rf_{t}")
                nc.tensor.matmul(rf[:, 0:512], wo_t[0:64, :], p2[0:64, :],
                                 start=True, stop=True, tile_position=(0, 0))
                nc.tensor.matmul(rf[:, 512:1024], wo_t[64:128, :], p2[64:128, :],
                                 start=True, stop=True, tile_position=(64, 0))

                res_t = ph3_p.tile([128, 1024], F16, tag="res")
                nc.sync.dma_start(res_t[:], resid16[:, 1024 * t:1024 * (t + 1)])
                p3_state[t] = (rf, res_t)

            def emit_p3b(t):
                # phase-3 final: residual add + relu + store, issued an
                # iteration after emit_p3a so the DVE ops land behind the next
                # segment-max instead of ahead of it.
                rf, res_t = p3_state.pop(t)
                o_sb = ph3_p.tile([128, 1024], F16, tag="osb")
                nc.vector.tensor_tensor(o_sb[:], rf[:], res_t[:], op=mybir.AluOpType.add)
                nc.vector.tensor_scalar_max(o_sb[:], o_sb[:], 0.0)
                nc.sync.dma_start(out_t[:, 1024 * t:1024 * (t + 1)], o_sb[:])

            # prologue
            f1n_stage(0)
            for s in range(S):
                if s % 2 == 0 and 2 * (s // 2 + 2) < S:
                    load_dfn(s // 2 + 2)
                if s % 4 == 2 and 4 * (s // 4 + 1) < S:
                    load_p8(s // 4 + 1)
                if s + 1 < S:
                    f1n_stage(s + 1)

                # z1 accumulation: 3 matmuls per quarter, round-robin across quarters
                z1 = psz.tile([128, 1024], F32, tag="z1", name=f"z1_{s}")
                p8_t = p8_tiles[s // 4]
                f1n_t = f1n_tiles.pop(s)
                pcol = 512 * (s % 4)
                for q in range(4):
                    tp = (32 * q, 64 * (q % 2))
                    o = z1[64 * (q % 2):64 * (q % 2) + 64, 512 * (q // 2):512 * (q // 2) + 512]
                    nc.tensor.matmul(o, wp4_t[32 * q:32 * q + 32, :],
                                     p8_t[32 * q:32 * q + 32, pcol:pcol + 512],
                                     start=True, stop=False, tile_position=tp,
                                     skip_group_check=True)
                for q in range(4):
                    tp = (32 * q, 64 * (q % 2))
                    o = z1[64 * (q % 2):64 * (q % 2) + 64, 512 * (q // 2):512 * (q // 2) + 512]
                    rhs = f1pk[32 * q:32 * q + 32, 16 * s:16 * s + 16].rearrange(
                        "p (d one) -> p d one", one=1).to_broadcast([32, 16, 32])
                    nc.tensor.matmul(o, wc4_t[32 * q:32 * q + 32, :], rhs,
                                     start=False, stop=False, tile_position=tp,
                                     skip_group_check=True)
                for q in range(4):
                    tp = (32 * q, 64 * (q % 2))
                    o = z1[64 * (q % 2):64 * (q % 2) + 64, 512 * (q // 2):512 * (q // 2) + 512]
                    nc.tensor.matmul(o, wn4_t[32 * q:32 * q + 32, :],
                                     f1n_t[32 * q:32 * q + 32, :],
                                     start=False, stop=True, tile_position=tp,
                                     skip_group_check=True)

                h1 = h_p.tile([128, 1024], F16, tag="h1", name=f"h1_{s}")
                nc.scalar.activation(h1[:], z1[:], RELU, bias=bp0_t[:], scale=1.0)

                h1_tiles[s] = h1
                if s >= 2:
                    emit_l2(s - 2, h1_tiles.pop(s - 2))
                # phase-3 tile t's pooled cols complete at s = 16t+17; fire at
                # 16t+28 so the ACT-queue wait on segmax has ~11 supertiles of
                # slack (tiles 5/6 drain after the loop).
                if s >= 28 and (s - 28) % 16 == 0 and (s - 28) // 16 <= 4:
                    emit_p3a((s - 28) // 16)
                if s >= 29 and (s - 29) % 16 == 0 and (s - 29) // 16 <= 4:
                    emit_p3b((s - 29) // 16)
            emit_l2(S - 2, h1_tiles.pop(S - 2))
            emit_p3a(PT3 - 2)
            emit_l2(S - 1, h1_tiles.pop(S - 1))
            emit_p3b(PT3 - 2)
            emit_p3a(PT3 - 1)
            emit_p3b(PT3 - 1)

    nc.compile()
    return nc


def _dets_of_core(k):
    return np.arange(DC_REAL * k, DC_REAL * (k + 1))


def _host_prep(detFeatures, cIdxs, nIdxs, pairFeatures,
               W1, b1, Wp0, bp0, Wp1, bp1, Wq0, bq0, Wq1, bq1, Wo, bo):
    """Build per-core input maps. Returns (in_maps, out_perm) where out_perm
    maps device output columns back to det order."""
    f16 = np.float16
    detF = np.asarray(detFeatures, np.float32)
    pairF = np.asarray(pairFeatures, np.float32)
    nI = np.asarray(nIdxs, np.int64)

    # weights (shared across cores), packed per dtype into single tensors
    W1_32 = np.ascontiguousarray(W1, np.float32)
    W1_16 = W1_32.astype(f16)                                          # [128, 32]
    wpk16 = np.concatenate([
        W1_16,
        np.tile(Wp0[32:64].astype(f16), (4, 1)),                       # wc4
        np.tile(Wp0[64:96].astype(f16), (4, 1)),                       # wn4
        np.tile(Wp1.astype(f16), (2, 1)),                              # wp1_2
        np.tile(Wq0.astype(f16), (2, 1)),                              # wq0_2
        np.tile(Wq1.astype(f16), (2, 1)),                              # wq1_2
        np.tile(Wo.astype(f16), (2, 1)),                               # wo2
    ], axis=1)                                                         # [128, 480]
    wpk8 = np.concatenate([
        W1_32.astype(NP8),
        np.tile(Wp0[0:32].astype(NP8), (4, 1)),                        # wp4
    ], axis=1)                                                         # [128, 96]
    bpk = np.stack([
        np.tile(np.asarray(b1, np.float32), 4),
        np.tile(np.asarray(bp0, np.float32), 2),
        np.tile(np.asarray(bp1, np.float32), 2),
        np.tile(np.asarray(bq0, np.float32), 2),
        np.tile(np.asarray(bq1, np.float32), 2),
    ], axis=1)                                                         # [128, 5]
    bo32 = np.asarray(bo, np.float32)

    # det-order scramble for pooled/output columns:
    # local det d: s = d//64, q = (d%64)//16, i = d%16
    d = np.arange(DC)
    s_, q_, i_ = d // 64, (d % 64) // 16, d % 16
    pooled_col = 32 * s_ + 16 * (q_ // 2) + i_
    half = q_ % 2
    t3 = pooled_col // 512
    out_col = 1024 * t3 + 512 * half + (pooled_col % 512)              # [DC]

    detF8u = detF.astype(NP8).view(np.uint8)                           # [N, 128]

    in_maps = []
    for k in range(N_CORES):
        dets = _dets_of_core(k)
        dloc = detF[dets]                                              # [6250, 128]
        dpad = np.zeros((DC, 128), np.float32)
        dpad[:DC_REAL] = dloc
        detft16 = np.ascontiguousarray(dpad.T.astype(f16))             # [128, DC]

        # resid16[:, out_col[d]] = detF[d] + bo  (scrambled; pads zero)
        resid = np.zeros((2 * PCP, 128), np.float32)
        resid[out_col[:DC_REAL]] = dloc + bo32
        resid16 = np.ascontiguousarray(resid.T.astype(f16))            # [128, 2*PCP]

        # pairs of this core, padded; fp8 strip packing as in the f16 version
        pf = np.zeros((PAIRS, 32), NP8)
        pf[:DC_REAL * KN] = pairF[DC_REAL * KN * k: DC_REAL * KN * (k + 1)].astype(NP8)
        p8 = np.ascontiguousarray(
            pf.reshape(S, 4, 512, 32).transpose(1, 3, 0, 2).reshape(128, S * 512)
        )

        ni = np.zeros(PAIRS, np.int64)
        ni[:DC_REAL * KN] = nI[DC_REAL * KN * k: DC_REAL * KN * (k + 1)]
        detfn8 = np.ascontiguousarray(detF8u[ni].T).view(NP8)          # [128, PAIRS]

        in_maps.append({
            "detft16": detft16, "p8": p8.view(NP8), "detfn8": detfn8,
            "resid16": resid16,
            "wpk16": wpk16, "wpk8": wpk8, "bpk": bpk,
        })
    return in_maps, out_col


def _run(inputs, trace=False):
    if "nc" not in _CACHE:
        _CACHE["nc"] = _build()
    nc = _CACHE["nc"]
    in_maps, out_col = _host_prep(**inputs)
    res = run_bass_kernel_spmd(nc, in_maps, core_ids=list(range(N_CORES)),
                               trace=trace)
    outs = []
    for k in range(N_CORES):
        ot = res.results[k]["out_t"]                                   # [128, 2*PCP]
        outs.append(ot[:, out_col[:DC_REAL]].T)                        # [6250, 128]
    full = np.concatenate(outs, axis=0).astype(np.float32)
    return full, res


def kernel(**inputs):
    inputs = {k: np.asarray(v) for k, v in inputs.items()}
    full, _ = _run(inputs, trace=False)
    return full
